# revision 22
# baseline (speedup 1.0000x reference)
"""Trainium2 Bass kernel for nn_AttentionBlock (B=4, C=64, H=W=64, INTER=8).

Sharding: 8 cores = 4 batches x 2 query-halves. Each core computes, for its
batch b and its half of the query pixels (n), the full attention output
gamma * (V @ softmax(Q^T K)^T) + x over all m=4096 keys. The host permutes
each core's pixel columns so the core's own query half sits first (attention
is permutation-invariant over keys), so every core runs the same program.

Design notes (all timings from the CoreSim cost model this target runs on):

  - Quadratic-form energy: e[n,m] = x_aug_n^T M x_aug_m with
    M = [[Wq^T Wk, Wq^T bk], [bq^T Wk, bq^T bk]] / 64 folded host-side.
    On-device setup is one z = M^T x_aug pass (8 matmuls + 8 PSUM->SBUF
    copies); energy blocks are then zt-stationary x_aug-moving matmuls, so
    the moving operand comes straight from the DMA'd input (no q/k copies).

  - The exp stream is split across BOTH elementwise engines:
      * ACT: exp(64*u) via activation scale=64 (exact table exp),
        ~1004ns per 1024-elem group.
      * DVE: two registered custom-DVE ops (registered into
        concourse.dve_ops at import): EXPG_P1 = deg-4 Horner poly for
        exp(u) (coeffs fitted on |e|<=24, c0=c1=1), EXPG_P2 = six chained
        squarings -> exp(u)^64 = exp(e). Max rel err ~4e-4 over the actual
        energy range (|e| < 17), below the bf16 output rounding.
    Groups are assigned to engines per-window to balance (ACT ~1.0us per
    group vs DVE ~2.3us; DVE also owns the setup copies + epilogue).

  - AV is TRANSPOSED: oa[n-block 128, 65] += exp(E)[m,n-block]^T @ vT[m,65]
    per m-block, i.e. 65-cycle moving operands (~36ns each, 128/window)
    instead of 512-cycle [65,512] accumulations (~216ns, 32/window).
    Stationary loads are not charged by the cost model. Column 64 of vT is
    ones -> oa column 64 accumulates the softmax denominator.
    All 4 accumulators of a window pack into ONE PSUM bank; only the very
    first AV matmul of a window uses start=True (start zeroes the whole
    2KB bank), everything else accumulates.

  - Epilogue per window: reciprocal_approx_fast on the 4 denominators
    (strided [128,4] view), then one affine_then_add per n-block
    (out = oa*rec + xres), DMA out in [n, c] layout (host untransposes).

  - PSUM: 8 banks = energy 2x[128,1024] + oa 2x[128,260] + setup/warmup
    2x[128,512]. PE clock ramps only after ~3us of continuous busy; a short
    warmup stream bridges the z0-copy wait at the head.
"""

import os
import sys
import types
import numpy as np
import ml_dtypes


def _ensure_ntff_hook_importable():
    """bass_utils imports antenv.axon_hooks when tracing is requested via
    BASS_TRACE; some images lack that module. Provide it (backed by the
    ctypes hook from trn_boot when available, else a None hook, which
    bass_utils handles by skipping the trace)."""
    try:
        import antenv.axon_hooks  # noqa: F401
        return
    except ImportError:
        pass
    hook = None
    try:
        from trn_agent_boot.trn_boot import _ntff_profile_via_ctypes
        so = "/opt/axon/libaxon_pjrt.so"
        if os.path.exists(so):
            hook = _ntff_profile_via_ctypes(so)
    except Exception:
        hook = None
    mod = types.ModuleType("antenv.axon_hooks")
    mod.get_axon_ntff_profile_hook = lambda: hook
    sys.modules["antenv.axon_hooks"] = mod


B, C, H, W = 4, 64, 64, 64
N = H * W              # 4096 pixels
NHALF = N // 2         # 2048 query pixels per core
NCORES = 8
MBLK = 128             # m-block (PSUM partition tile)
WIN = 512              # query-window width
NWIN = NHALF // WIN    # 4
NJ = N // MBLK         # 32 m-blocks
GRP = 2                # m-blocks per exp group
NGRP = NJ // GRP       # 16 groups per window
XW = 192               # weights prefix cols in xall (M_padT 128 + Wv_aug 64)
XCOLS = XW + N

ESCALE = 64.0          # energy prescale folded into M host-side
# deg-4 poly coeffs for exp(u) on |u| <= 24/64, c0=c1=1 (fit min rel err)
PC4, PC3, PC2 = 0.04083403291898538, 0.1675708986424995, 0.5000982898691694

# per-window ACT-group counts (rest of the 16 are DVE groups), tunable
ACT_COUNTS = [int(v) for v in os.environ.get("KACT", "14,11,11,12").split(",")]
LAG_ACT = int(os.environ.get("KLAGA", "1"))
LAG_DVE = int(os.environ.get("KLAGD", "3"))
NWARM = int(os.environ.get("KWARM", "0"))

_compiled = {}
LAST_RESULT = None


def _dve_positions(nact, first=False, last=False):
    """Spread the (16 - nact) DVE groups evenly over slots [lo, 13]: slots
    0-1 stay ACT so the window's first AVs flush early, and even spacing
    keeps both exp engines continuously fed (clustering starves one of
    them). Window 0 starts at slot 5 — its early slots are congested with
    the dripped setup pieces, and an early DVE group there holds an eps
    slot hostage."""
    nd = NGRP - nact
    if nd <= 0:
        return set()
    lo, hi = (5, 13) if first else ((2, 11) if last else (2, 13))
    pos = set()
    for i in range(nd):
        p = lo + int(round(i * (hi - lo) / max(1, nd - 1))) if nd > 1 \
            else (lo + hi) // 2
        while p in pos:
            p += 1
        pos.add(min(p, NGRP - 1))
    return pos


def _register_exp_ops():
    """Register the two exp custom-DVE ops into concourse.dve_ops so both
    the CoreSim reference execution and the per-NEFF uop-table generation
    (bass_utils.dve_table_for_ops) can see them."""
    import concourse.dve_ops as dops
    from concourse.dve_spec import Spec, Src0, C0, C1, C2, One, lower
    from concourse.dve_uop import DveOpSpec

    def p1_ref(in0, in1, s0, s1, imm2):
        x = in0.astype(np.float32)
        one = np.float32(1.0)
        return ((((x * np.float32(s0) + np.float32(s1)) * x + np.float32(imm2))
                 * x + one) * x + one).astype(np.float32)

    def p2_ref(in0, in1, s0, s1, imm2):
        x = in0.astype(np.float32)
        for _ in range(6):
            x = (x * x).astype(np.float32)
        return x

    u = Src0
    p1_body = ((((u * C0 + C1) * u + C2) * u + One) * u + One)
    t = Src0
    for _ in range(6):
        t = t * t

    ops = {}
    for name, body, ref in [("ANT_EXPG_P1", p1_body, p1_ref),
                            ("ANT_EXPG_P2", t, p2_ref)]:
        existing = next((o for o in dops.OPS if o.name == name), None)
        if existing is not None:
            ops[name] = existing
            continue
        spec = Spec(body=body, reference=ref)
        row = dops._CUSTOM_DVE_ROW_BASE + len(dops.OPS)
        assert row < 0x20, "custom-DVE opcode rows exhausted"
        dops._SUB_OPCODE_FOR_NAME[name] = row
        sha = {}
        for ver in ("v3", "v4"):
            try:
                uops = lower(spec, ver=ver)
                sha[ver] = DveOpSpec(name=name, opcode=row, uops=uops,
                                     rd1_en=False).sha(ver)
            except Exception:
                pass
        op = dops.DveOp(name, spec, subdim=False, uops_sha=sha)
        dops.OPS.append(op)
        dops.CUSTOM_DVE_SPECS[name] = spec
        ops[name] = op
    return ops["ANT_EXPG_P1"], ops["ANT_EXPG_P2"]


def _build():
    import concourse.bacc as bacc
    import concourse.mybir as mybir
    from concourse.tile import TileContext

    EXP_P1, EXP_P2 = _register_exp_ops()

    dt = mybir.dt
    f32, bf16 = dt.float32, dt.bfloat16
    EXP = mybir.ActivationFunctionType.Exp

    nc = bacc.Bacc("TRN2", target_bir_lowering=False, debug=False,
                   num_devices=NCORES)

    xall = nc.dram_tensor("xall", [128, XCOLS], bf16, kind="ExternalInput").ap()
    xres = nc.dram_tensor("xres", [128, 16 * C], f32, kind="ExternalInput").ap()
    outt = nc.dram_tensor("outt", [128, 16 * C], f32, kind="ExternalOutput").ap()

    with TileContext(nc) as tc:
        with tc.tile_pool(name="const", bufs=1) as cp, \
             tc.tile_pool(name="eps", bufs=3, space="PSUM") as eps, \
             tc.tile_pool(name="oap", bufs=2, space="PSUM") as oap, \
             tc.tile_pool(name="wp", bufs=6) as wp, \
             tc.tile_pool(name="pp", bufs=2) as pp, \
             tc.tile_pool(name="fp", bufs=2) as fp:

            # ---- DMA in: weights + window-0 block first, then the rest ----
            # DMA: weights + window-0 queries/first keys, then keys 512:2048
            # (unblocks the z1-z3 setup drips early), then the second half
            # d1 on the SP queue; d2a/d2b on the Activation HWDGE queue so
            # the descriptor generations run concurrently, not serially
            xall_t = cp.tile([128, XCOLS], bf16, tag="xa", name="xall_t")
            nc.scalar.dma_start(out=xall_t[:, 0:XW + WIN],
                                in_=xall[:, 0:XW + WIN])
            nc.scalar.dma_start(out=xall_t[:, XW + WIN:XW + 4 * WIN],
                                in_=xall[:, XW + WIN:XW + 4 * WIN])
            nc.scalar.dma_start(out=xall_t[:, XW + 4 * WIN:],
                                in_=xall[:, XW + 4 * WIN:])
            xr_t = cp.tile([128, 16 * C], f32, tag="xr", name="xr_t")

            zt = cp.tile([128, N], bf16, tag="z", name="zt")
            vt = cp.tile([128, NJ * (C + 1)], bf16, tag="vt", name="vt")
            vt3 = vt.rearrange("p (j c) -> p j c", c=C + 1)
            nc.vector.memset(vt3[:, :, C], 1.0)

            if NWARM > 0:
                wu = cp.tile([128, WIN], bf16, tag="wu", name="wu")
                nc.vector.memset(wu[:, :], 0.0)

            x_aug = xall_t[:, XW:]          # [128, 4096] keys/queries
            wv_aug = xall_t[:, 128:XW]      # [128, 64]

            # ---- setup emitters ----
            def emit_z(c):
                # z chunk c: [128, 512] = M_padT^T-applied x_aug columns
                zp = eps.tile([128, GRP * WIN], f32, tag="e", name="zp")
                nc.tensor.matmul(zp[:, 0:WIN], xall_t[:, 0:128],
                                 x_aug[:, WIN * c:WIN * (c + 1)],
                                 start=True, stop=True)
                nc.vector.tensor_copy(zt[:, WIN * c:WIN * (c + 1)],
                                      zp[:, 0:WIN])

            def emit_vt(p8):
                # vT blocks 8p8..8p8+7: out[m,64] per block, one 512-col
                # copy per 8 blocks (all within one PSUM bank; the per-block
                # start=True lazy-zero does not disturb sibling blocks)
                vp = eps.tile([128, GRP * WIN], f32, tag="e", name="vp")
                for jj in range(8):
                    j = 8 * p8 + jj
                    nc.tensor.matmul(vp[:, C * jj:C * (jj + 1)],
                                     x_aug[:, MBLK * j:MBLK * (j + 1)],
                                     wv_aug, start=True, stop=True)
                vp8 = vp.rearrange("p (j c) -> p j c", c=C)
                nc.vector.tensor_copy(vt3[:, 8 * p8:8 * p8 + 8, 0:C],
                                      vp8[:, 0:8, :])

            # eager: z0 + v0 (window 0 group 0/1 needs them)
            emit_z(0)
            emit_vt(0)

            # optional warmup matmuls (into an eps slot) to start the PE
            # clock ramp while the z0 copy is in flight
            if NWARM > 0:
                wu_p = eps.tile([128, GRP * WIN], f32, tag="e", name="wu_p")
                for _ in range(NWARM):
                    nc.tensor.matmul(wu_p[:, 0:WIN], wu[:, 0:128], wu[:, :],
                                     start=True, stop=True)

            # interleaved drip for the remaining setup: z_c before group 2c's
            # energy (emitted at iteration 2c-1), vT piece p (8 blocks)
            # before its AVs flush (~group 4p)
            setup_thunks = [lambda: emit_z(1), lambda: emit_vt(1),
                            lambda: emit_z(2), lambda: emit_z(3),
                            lambda: emit_vt(2), lambda: emit_z(4),
                            lambda: emit_z(5), lambda: emit_vt(3),
                            lambda: emit_z(6), lambda: emit_z(7)]

            # ---- windows ----
            pend_av = []   # (oa, ex, j0, flush_at, first)

            def flush_one(last=False):
                oa_p, ex_p, j0, _fa, first = pend_av.pop(0)
                for jj in range(GRP):
                    for b in range(4):
                        nc.tensor.matmul(
                            oa_p[:, 65 * b:65 * (b + 1)],
                            ex_p[:, 512 * jj + 128 * b:512 * jj + 128 * (b + 1)],
                            vt3[:, j0 + jj, :],
                            start=(first and jj == 0 and b == 0),
                            stop=(last and jj == GRP - 1),
                            skip_group_check=True)

            def epilogue(oa, w):
                oa3 = oa.rearrange("p (b c) -> p b c", c=65)
                rec = fp.tile([128, 4], f32, tag="rec", name="rec")
                nc.vector.reciprocal_approx_fast(out=rec[:, :],
                                                 in_=oa3[:, :, C])
                osb = fp.tile([128, 4 * C], f32, tag="osb", name="osb")
                for b in range(4):
                    nc.vector.affine_then_add(
                        out=osb[:, C * b:C * (b + 1)],
                        in0=oa3[:, b, 0:C],
                        in1=xr_t[:, 4 * C * w + C * b:4 * C * w + C * (b + 1)],
                        scale=rec[:, b:b + 1], bias=0.0)
                nc.sync.dma_start(out=outt[:, 4 * C * w:4 * C * (w + 1)],
                                  in_=osb[:, :])

            energy_tiles = {}

            def emit_energy(g, q_rhs):
                e = eps.tile([128, GRP * WIN], f32, tag="e", name="e")
                for jj in range(GRP):
                    j = GRP * g + jj
                    nc.tensor.matmul(e[:, WIN * jj:WIN * (jj + 1)],
                                     zt[:, MBLK * j:MBLK * (j + 1)],
                                     q_rhs, start=True, stop=True)
                energy_tiles[g] = e

            oa_prev = None
            for w in range(NWIN):
                dve_pos = _dve_positions(ACT_COUNTS[w], first=(w == 0),
                                         last=(w == NWIN - 1))
                oa = oap.tile([128, 4 * 65], f32, tag="oa", name=f"oa{w}")
                q_rhs = x_aug[:, WIN * w:WIN * (w + 1)]
                for g in range(NGRP):
                    is_dve = g in dve_pos
                    # energy runs one group ahead of exp so neither exp
                    # engine ever waits on the in-order PE stream
                    if g == 0:
                        emit_energy(0, q_rhs)
                    if g + 1 < NGRP:
                        emit_energy(g + 1, q_rhs)
                    e = energy_tiles.pop(g)
                    ex = wp.tile([128, GRP * WIN], bf16, tag="ex", name="ex")
                    if is_dve:
                        p = pp.tile([128, GRP * WIN], f32, tag="p", name="p")
                        nc.vector._custom_dve(EXP_P1, out=p[:, :], in0=e[:, :],
                                              s0=PC4, s1=PC3, imm2=PC2)
                        nc.vector._custom_dve(EXP_P2, out=ex[:, :], in0=p[:, :])
                    else:
                        nc.scalar.activation(ex[:, :], e[:, :], EXP,
                                             scale=ESCALE)
                    # drip one setup piece (window 0 only), after exp so the
                    # copy isn't queued ahead of exp work on the DVE
                    if setup_thunks:
                        setup_thunks.pop(0)()
                    pend_av.append((oa, ex, GRP * g,
                                    g + (LAG_DVE if is_dve else LAG_ACT),
                                    g == 0))
                    while pend_av and pend_av[0][3] <= g:
                        flush_one()
                    # window w-1's epilogue once its AVs are all flushed
                    if oa_prev is not None and g == 1:
                        epilogue(*oa_prev)
                        oa_prev = None
                    # the residual DMA is only needed by window 0's epilogue;
                    # issuing it late keeps the input-DMA head short
                    if w == 0 and g == 2:
                        nc.sync.dma_start(out=xr_t[:, :], in_=xres)
                # flush the window's remaining AVs
                while pend_av:
                    flush_one(last=(len(pend_av) == 1))
                oa_prev = (oa, w)
            epilogue(*oa_prev)

    nc.compile()
    return nc


def _get_compiled():
    if "nc" not in _compiled:
        _compiled["nc"] = _build()
    return _compiled["nc"]


def kernel(x, Wq, bq, Wk, bk, Wv, bv, gamma):
    global LAST_RESULT
    _ensure_ntff_hook_importable()
    from concourse.bass_utils import run_bass_kernel_spmd

    nc = _get_compiled()

    x = np.asarray(x, dtype=np.float32)
    xf = x.reshape(B, C, N)
    Wq, Wk, Wv = np.asarray(Wq), np.asarray(Wk), np.asarray(Wv)
    bq, bk, bv = np.asarray(bq), np.asarray(bk), np.asarray(bv)
    gval = float(np.asarray(gamma).reshape(-1)[0])

    # quadratic-form energy matrix (augmented with biases), prescaled by 1/64
    M65 = np.zeros((65, 65), np.float64)
    M65[0:C, 0:C] = Wq.T.astype(np.float64) @ Wk.astype(np.float64)
    M65[0:C, C] = Wq.T.astype(np.float64) @ bk.astype(np.float64)
    M65[C, 0:C] = bq.astype(np.float64) @ Wk.astype(np.float64)
    M65[C, C] = float(bq.astype(np.float64) @ bk.astype(np.float64))
    m_padt = np.zeros((128, 128), np.float32)
    m_padt[0:65, 0:65] = (M65 / ESCALE).T.astype(np.float32)

    wv_aug = np.zeros((128, C), np.float32)
    wv_aug[0:C] = gval * Wv.T
    wv_aug[C] = gval * bv

    in_maps = []
    for core in range(NCORES):
        b, h = divmod(core, 2)
        own = xf[b][:, h * NHALF:(h + 1) * NHALF]
        oth = xf[b][:, (1 - h) * NHALF:(2 - h) * NHALF]
        x_aug = np.zeros((128, N), np.float32)
        x_aug[0:C] = np.concatenate([own, oth], axis=1)
        x_aug[C] = 1.0
        xall_core = np.concatenate([m_padt, wv_aug, x_aug], axis=1)
        # residual in [n-block, c] layout: xres_t[p, 64b+c] = own[c, 128b+p]
        xres_core = np.ascontiguousarray(
            own.reshape(C, 16, 128).transpose(2, 1, 0).reshape(128, 16 * C),
            dtype=np.float32)
        in_maps.append({
            "xall": np.ascontiguousarray(xall_core.astype(ml_dtypes.bfloat16)),
            "xres": xres_core,
        })

    trace = bool(os.environ.get("KTRACE"))
    res = run_bass_kernel_spmd(nc, in_maps, list(range(NCORES)), trace=trace)
    LAST_RESULT = res

    outf = np.empty((B, C, N), dtype=np.float32)
    for core in range(NCORES):
        b, h = divmod(core, 2)
        r = res.results[core]["outt"]          # [128, 16*64]
        half = r.reshape(128, 16, C).transpose(2, 1, 0).reshape(C, NHALF)
        outf[b][:, h * NHALF:(h + 1) * NHALF] = half
    return outf.reshape(B, C, H, W)


# revision 23
# speedup vs baseline: 1.0456x; 1.0456x over previous
"""Trainium2 Bass kernel for nn_AttentionBlock (B=4, C=64, H=W=64, INTER=8).

Sharding: 8 cores = 4 batches x 2 query-halves. Each core computes, for its
batch b and its half of the query pixels (n), the full attention output
gamma * (V @ softmax(Q^T K)^T) + x over all m=4096 keys. The host permutes
each core's pixel columns so the core's own query half sits first (attention
is permutation-invariant over keys), so every core runs the same program.

Design notes (all timings from the CoreSim cost model this target runs on):

  - Quadratic-form energy: e[n,m] = x_aug_n^T M x_aug_m with
    M = [[Wq^T Wk, Wq^T bk], [bq^T Wk, bq^T bk]] / 64 folded host-side.
    On-device setup is one z = M^T x_aug pass (8 matmuls + 8 PSUM->SBUF
    copies); energy blocks are then zt-stationary x_aug-moving matmuls, so
    the moving operand comes straight from the DMA'd input (no q/k copies).

  - The exp stream is split across BOTH elementwise engines:
      * ACT: exp(64*u) via activation scale=64 (exact table exp),
        ~1004ns per 1024-elem group.
      * DVE: two registered custom-DVE ops (registered into
        concourse.dve_ops at import): EXPG_P1 = deg-4 Horner poly for
        exp(u) (coeffs fitted on |e|<=24, c0=c1=1), EXPG_P2 = six chained
        squarings -> exp(u)^64 = exp(e). Max rel err ~4e-4 over the actual
        energy range (|e| < 17), below the bf16 output rounding.
    Groups are assigned to engines per-window to balance (ACT ~1.0us per
    group vs DVE ~2.3us; DVE also owns the setup copies + epilogue).

  - AV is TRANSPOSED: oa[n-block 128, 65] += exp(E)[m,n-block]^T @ vT[m,65]
    per m-block, i.e. 65-cycle moving operands (~36ns each, 128/window)
    instead of 512-cycle [65,512] accumulations (~216ns, 32/window).
    Stationary loads are not charged by the cost model. Column 64 of vT is
    ones -> oa column 64 accumulates the softmax denominator.
    All 4 accumulators of a window pack into ONE PSUM bank; only the very
    first AV matmul of a window uses start=True (start zeroes the whole
    2KB bank), everything else accumulates.

  - Epilogue per window: reciprocal_approx_fast on the 4 denominators
    (strided [128,4] view), then one affine_then_add per n-block
    (out = oa*rec + xres), DMA out in [n, c] layout (host untransposes).

  - PSUM: 8 banks = energy 2x[128,1024] + oa 2x[128,260] + setup/warmup
    2x[128,512]. PE clock ramps only after ~3us of continuous busy; a short
    warmup stream bridges the z0-copy wait at the head.
"""

import os
import sys
import types
import numpy as np
import ml_dtypes


def _ensure_ntff_hook_importable():
    """bass_utils imports antenv.axon_hooks when tracing is requested via
    BASS_TRACE; some images lack that module. Provide it (backed by the
    ctypes hook from trn_boot when available, else a None hook, which
    bass_utils handles by skipping the trace)."""
    try:
        import antenv.axon_hooks  # noqa: F401
        return
    except ImportError:
        pass
    hook = None
    try:
        from trn_agent_boot.trn_boot import _ntff_profile_via_ctypes
        so = "/opt/axon/libaxon_pjrt.so"
        if os.path.exists(so):
            hook = _ntff_profile_via_ctypes(so)
    except Exception:
        hook = None
    mod = types.ModuleType("antenv.axon_hooks")
    mod.get_axon_ntff_profile_hook = lambda: hook
    sys.modules["antenv.axon_hooks"] = mod


B, C, H, W = 4, 64, 64, 64
N = H * W              # 4096 pixels
NHALF = N // 2         # 2048 query pixels per core
NCORES = 8
MBLK = 128             # m-block (PSUM partition tile)
WIN = 512              # query-window width
NWIN = NHALF // WIN    # 4
NJ = N // MBLK         # 32 m-blocks
GRP = 2                # m-blocks per exp group
NGRP = NJ // GRP       # 16 groups per window
XW = 192               # weights prefix cols in xall (M_padT 128 + Wv_aug 64)
XCOLS = XW + N

ESCALE = 64.0          # energy prescale folded into M host-side
# deg-4 poly coeffs for exp(u) on |u| <= 24/64, c0=c1=1 (fit min rel err)
PC4, PC3, PC2 = 0.04083403291898538, 0.1675708986424995, 0.5000982898691694

# per-window ACT-group counts (rest of the 16 are DVE groups), tunable
ACT_COUNTS = [int(v) for v in os.environ.get("KACT", "14,11,11,12").split(",")]
LAG_ACT = int(os.environ.get("KLAGA", "1"))
LAG_DVE = int(os.environ.get("KLAGD", "3"))
NWARM = int(os.environ.get("KWARM", "0"))

_compiled = {}
LAST_RESULT = None


def _dve_positions(nact, first=False, last=False):
    """Spread the (16 - nact) DVE groups evenly over slots [lo, 13]: slots
    0-1 stay ACT so the window's first AVs flush early, and even spacing
    keeps both exp engines continuously fed (clustering starves one of
    them). Window 0 starts at slot 5 — its early slots are congested with
    the dripped setup pieces, and an early DVE group there holds an eps
    slot hostage."""
    nd = NGRP - nact
    if nd <= 0:
        return set()
    lo, hi = (5, 13) if first else ((2, 11) if last else (2, 13))
    pos = set()
    for i in range(nd):
        p = lo + int(round(i * (hi - lo) / max(1, nd - 1))) if nd > 1 \
            else (lo + hi) // 2
        while p in pos:
            p += 1
        pos.add(min(p, NGRP - 1))
    return pos


def _register_exp_ops():
    """Register the two exp custom-DVE ops into concourse.dve_ops so both
    the CoreSim reference execution and the per-NEFF uop-table generation
    (bass_utils.dve_table_for_ops) can see them."""
    import concourse.dve_ops as dops
    from concourse.dve_spec import Spec, Src0, C0, C1, C2, One, lower
    from concourse.dve_uop import DveOpSpec

    def p1_ref(in0, in1, s0, s1, imm2):
        x = in0.astype(np.float32)
        one = np.float32(1.0)
        return ((((x * np.float32(s0) + np.float32(s1)) * x + np.float32(imm2))
                 * x + one) * x + one).astype(np.float32)

    def p2_ref(in0, in1, s0, s1, imm2):
        x = in0.astype(np.float32)
        for _ in range(6):
            x = (x * x).astype(np.float32)
        return x

    u = Src0
    p1_body = ((((u * C0 + C1) * u + C2) * u + One) * u + One)
    t = Src0
    for _ in range(6):
        t = t * t

    ops = {}
    for name, body, ref in [("ANT_EXPG_P1", p1_body, p1_ref),
                            ("ANT_EXPG_P2", t, p2_ref)]:
        existing = next((o for o in dops.OPS if o.name == name), None)
        if existing is not None:
            ops[name] = existing
            continue
        spec = Spec(body=body, reference=ref)
        row = dops._CUSTOM_DVE_ROW_BASE + len(dops.OPS)
        assert row < 0x20, "custom-DVE opcode rows exhausted"
        dops._SUB_OPCODE_FOR_NAME[name] = row
        sha = {}
        for ver in ("v3", "v4"):
            try:
                uops = lower(spec, ver=ver)
                sha[ver] = DveOpSpec(name=name, opcode=row, uops=uops,
                                     rd1_en=False).sha(ver)
            except Exception:
                pass
        op = dops.DveOp(name, spec, subdim=False, uops_sha=sha)
        dops.OPS.append(op)
        dops.CUSTOM_DVE_SPECS[name] = spec
        ops[name] = op
    return ops["ANT_EXPG_P1"], ops["ANT_EXPG_P2"]


def _build():
    import concourse.bacc as bacc
    import concourse.mybir as mybir
    from concourse.tile import TileContext

    EXP_P1, EXP_P2 = _register_exp_ops()

    dt = mybir.dt
    f32, bf16 = dt.float32, dt.bfloat16
    EXP = mybir.ActivationFunctionType.Exp

    nc = bacc.Bacc("TRN2", target_bir_lowering=False, debug=False,
                   num_devices=NCORES)

    xall = nc.dram_tensor("xall", [128, XCOLS], bf16, kind="ExternalInput").ap()
    xres = nc.dram_tensor("xres", [128, 16 * C], f32, kind="ExternalInput").ap()
    outt = nc.dram_tensor("outt", [128, 16 * C], f32, kind="ExternalOutput").ap()

    with TileContext(nc) as tc:
        with tc.tile_pool(name="const", bufs=1) as cp, \
             tc.tile_pool(name="eps", bufs=3, space="PSUM") as eps, \
             tc.tile_pool(name="oap", bufs=2, space="PSUM") as oap, \
             tc.tile_pool(name="wp", bufs=6) as wp, \
             tc.tile_pool(name="pp", bufs=2) as pp, \
             tc.tile_pool(name="fp", bufs=2) as fp:

            # ---- DMA in: weights + window-0 block first, then the rest ----
            # DMA: weights + window-0 queries/first keys, then keys 512:2048
            # (unblocks the z1-z3 setup drips early), then the second half
            # d1 on the SP queue; d2a/d2b on the Activation HWDGE queue so
            # the descriptor generations run concurrently, not serially
            xall_t = cp.tile([128, XCOLS], bf16, tag="xa", name="xall_t")
            nc.sync.dma_start(out=xall_t[:, 0:XW + WIN], in_=xall[:, 0:XW + WIN])
            nc.scalar.dma_start(out=xall_t[:, XW + WIN:XW + 4 * WIN],
                                in_=xall[:, XW + WIN:XW + 4 * WIN])
            nc.scalar.dma_start(out=xall_t[:, XW + 4 * WIN:],
                                in_=xall[:, XW + 4 * WIN:])
            xr_t = cp.tile([128, 16 * C], f32, tag="xr", name="xr_t")

            zt = cp.tile([128, N], bf16, tag="z", name="zt")
            vt = cp.tile([128, NJ * (C + 1)], bf16, tag="vt", name="vt")
            vt3 = vt.rearrange("p (j c) -> p j c", c=C + 1)
            nc.vector.memset(vt3[:, :, C], 1.0)

            if NWARM > 0:
                wu = cp.tile([128, WIN], bf16, tag="wu", name="wu")
                nc.vector.memset(wu[:, :], 0.0)

            x_aug = xall_t[:, XW:]          # [128, 4096] keys/queries
            wv_aug = xall_t[:, 128:XW]      # [128, 64]

            # ---- setup emitters ----
            def emit_z(c):
                # z chunk c: [128, 512] = M_padT^T-applied x_aug columns
                zp = eps.tile([128, GRP * WIN], f32, tag="e", name="zp")
                nc.tensor.matmul(zp[:, 0:WIN], xall_t[:, 0:128],
                                 x_aug[:, WIN * c:WIN * (c + 1)],
                                 start=True, stop=True)
                nc.vector.tensor_copy(zt[:, WIN * c:WIN * (c + 1)],
                                      zp[:, 0:WIN])

            def emit_vt(p8):
                # vT blocks 8p8..8p8+7: out[m,64] per block, one 512-col
                # copy per 8 blocks (all within one PSUM bank; the per-block
                # start=True lazy-zero does not disturb sibling blocks)
                vp = eps.tile([128, GRP * WIN], f32, tag="e", name="vp")
                for jj in range(8):
                    j = 8 * p8 + jj
                    nc.tensor.matmul(vp[:, C * jj:C * (jj + 1)],
                                     x_aug[:, MBLK * j:MBLK * (j + 1)],
                                     wv_aug, start=True, stop=True)
                vp8 = vp.rearrange("p (j c) -> p j c", c=C)
                nc.vector.tensor_copy(vt3[:, 8 * p8:8 * p8 + 8, 0:C],
                                      vp8[:, 0:8, :])

            # eager: z0 + v0 (window 0 group 0/1 needs them)
            emit_z(0)
            emit_vt(0)

            # optional warmup matmuls (into an eps slot) to start the PE
            # clock ramp while the z0 copy is in flight
            if NWARM > 0:
                wu_p = eps.tile([128, GRP * WIN], f32, tag="e", name="wu_p")
                for _ in range(NWARM):
                    nc.tensor.matmul(wu_p[:, 0:WIN], wu[:, 0:128], wu[:, :],
                                     start=True, stop=True)

            # interleaved drip for the remaining setup: z_c before group 2c's
            # energy (emitted at iteration 2c-1), vT piece p (8 blocks)
            # before its AVs flush (~group 4p)
            setup_thunks = [lambda: emit_z(1), lambda: emit_vt(1),
                            lambda: emit_z(2), lambda: emit_z(3),
                            lambda: emit_vt(2), lambda: emit_z(4),
                            lambda: emit_z(5), lambda: emit_vt(3),
                            lambda: emit_z(6), lambda: emit_z(7)]

            # ---- windows ----
            pend_av = []   # (oa, ex, j0, flush_at, first)

            def flush_one(last=False):
                oa_p, ex_p, j0, _fa, first = pend_av.pop(0)
                for jj in range(GRP):
                    for b in range(4):
                        nc.tensor.matmul(
                            oa_p[:, 65 * b:65 * (b + 1)],
                            ex_p[:, 512 * jj + 128 * b:512 * jj + 128 * (b + 1)],
                            vt3[:, j0 + jj, :],
                            start=(first and jj == 0 and b == 0),
                            stop=(last and jj == GRP - 1),
                            skip_group_check=True)

            def epilogue(oa, w):
                oa3 = oa.rearrange("p (b c) -> p b c", c=65)
                rec = fp.tile([128, 4], f32, tag="rec", name="rec")
                nc.vector.reciprocal_approx_fast(out=rec[:, :],
                                                 in_=oa3[:, :, C])
                osb = fp.tile([128, 4 * C], f32, tag="osb", name="osb")
                for b in range(4):
                    nc.vector.affine_then_add(
                        out=osb[:, C * b:C * (b + 1)],
                        in0=oa3[:, b, 0:C],
                        in1=xr_t[:, 4 * C * w + C * b:4 * C * w + C * (b + 1)],
                        scale=rec[:, b:b + 1], bias=0.0)
                nc.sync.dma_start(out=outt[:, 4 * C * w:4 * C * (w + 1)],
                                  in_=osb[:, :])

            energy_tiles = {}

            def emit_energy(g, q_rhs):
                e = eps.tile([128, GRP * WIN], f32, tag="e", name="e")
                for jj in range(GRP):
                    j = GRP * g + jj
                    nc.tensor.matmul(e[:, WIN * jj:WIN * (jj + 1)],
                                     zt[:, MBLK * j:MBLK * (j + 1)],
                                     q_rhs, start=True, stop=True)
                energy_tiles[g] = e

            oa_prev = None
            for w in range(NWIN):
                dve_pos = _dve_positions(ACT_COUNTS[w], first=(w == 0),
                                         last=(w == NWIN - 1))
                oa = oap.tile([128, 4 * 65], f32, tag="oa", name=f"oa{w}")
                q_rhs = x_aug[:, WIN * w:WIN * (w + 1)]
                for g in range(NGRP):
                    is_dve = g in dve_pos
                    # energy runs one group ahead of exp so neither exp
                    # engine ever waits on the in-order PE stream
                    if g == 0:
                        emit_energy(0, q_rhs)
                    if g + 1 < NGRP:
                        emit_energy(g + 1, q_rhs)
                    e = energy_tiles.pop(g)
                    ex = wp.tile([128, GRP * WIN], bf16, tag="ex", name="ex")
                    if is_dve:
                        p = pp.tile([128, GRP * WIN], f32, tag="p", name="p")
                        nc.vector._custom_dve(EXP_P1, out=p[:, :], in0=e[:, :],
                                              s0=PC4, s1=PC3, imm2=PC2)
                        nc.vector._custom_dve(EXP_P2, out=ex[:, :], in0=p[:, :])
                    else:
                        nc.scalar.activation(ex[:, :], e[:, :], EXP,
                                             scale=ESCALE)
                    # drip one setup piece (window 0 only), after exp so the
                    # copy isn't queued ahead of exp work on the DVE
                    if setup_thunks:
                        setup_thunks.pop(0)()
                    pend_av.append((oa, ex, GRP * g,
                                    g + (LAG_DVE if is_dve else LAG_ACT),
                                    g == 0))
                    while pend_av and pend_av[0][3] <= g:
                        flush_one()
                    # window w-1's epilogue once its AVs are all flushed
                    if oa_prev is not None and g == 1:
                        epilogue(*oa_prev)
                        oa_prev = None
                    # the residual DMA is only needed by window 0's epilogue;
                    # issuing it late keeps the input-DMA head short
                    if w == 0 and g == 2:
                        nc.sync.dma_start(out=xr_t[:, :], in_=xres)
                # flush the window's remaining AVs
                while pend_av:
                    flush_one(last=(len(pend_av) == 1))
                oa_prev = (oa, w)
            epilogue(*oa_prev)

    nc.compile()
    return nc


def _get_compiled():
    if "nc" not in _compiled:
        _compiled["nc"] = _build()
    return _compiled["nc"]


def kernel(x, Wq, bq, Wk, bk, Wv, bv, gamma):
    global LAST_RESULT
    _ensure_ntff_hook_importable()
    from concourse.bass_utils import run_bass_kernel_spmd

    nc = _get_compiled()

    x = np.asarray(x, dtype=np.float32)
    xf = x.reshape(B, C, N)
    Wq, Wk, Wv = np.asarray(Wq), np.asarray(Wk), np.asarray(Wv)
    bq, bk, bv = np.asarray(bq), np.asarray(bk), np.asarray(bv)
    gval = float(np.asarray(gamma).reshape(-1)[0])

    # quadratic-form energy matrix (augmented with biases), prescaled by 1/64
    M65 = np.zeros((65, 65), np.float64)
    M65[0:C, 0:C] = Wq.T.astype(np.float64) @ Wk.astype(np.float64)
    M65[0:C, C] = Wq.T.astype(np.float64) @ bk.astype(np.float64)
    M65[C, 0:C] = bq.astype(np.float64) @ Wk.astype(np.float64)
    M65[C, C] = float(bq.astype(np.float64) @ bk.astype(np.float64))
    m_padt = np.zeros((128, 128), np.float32)
    m_padt[0:65, 0:65] = (M65 / ESCALE).T.astype(np.float32)

    wv_aug = np.zeros((128, C), np.float32)
    wv_aug[0:C] = gval * Wv.T
    wv_aug[C] = gval * bv

    in_maps = []
    for core in range(NCORES):
        b, h = divmod(core, 2)
        own = xf[b][:, h * NHALF:(h + 1) * NHALF]
        oth = xf[b][:, (1 - h) * NHALF:(2 - h) * NHALF]
        x_aug = np.zeros((128, N), np.float32)
        x_aug[0:C] = np.concatenate([own, oth], axis=1)
        x_aug[C] = 1.0
        xall_core = np.concatenate([m_padt, wv_aug, x_aug], axis=1)
        # residual in [n-block, c] layout: xres_t[p, 64b+c] = own[c, 128b+p]
        xres_core = np.ascontiguousarray(
            own.reshape(C, 16, 128).transpose(2, 1, 0).reshape(128, 16 * C),
            dtype=np.float32)
        in_maps.append({
            "xall": np.ascontiguousarray(xall_core.astype(ml_dtypes.bfloat16)),
            "xres": xres_core,
        })

    trace = bool(os.environ.get("KTRACE"))
    res = run_bass_kernel_spmd(nc, in_maps, list(range(NCORES)), trace=trace)
    LAST_RESULT = res

    outf = np.empty((B, C, N), dtype=np.float32)
    for core in range(NCORES):
        b, h = divmod(core, 2)
        r = res.results[core]["outt"]          # [128, 16*64]
        half = r.reshape(128, 16, C).transpose(2, 1, 0).reshape(C, NHALF)
        outf[b][:, h * NHALF:(h + 1) * NHALF] = half
    return outf.reshape(B, C, H, W)


# revision 24
# speedup vs baseline: 1.0907x; 1.0431x over previous
"""Trainium2 Bass kernel for nn_AttentionBlock (B=4, C=64, H=W=64, INTER=8).

Sharding: 8 cores = 4 batches x 2 query-halves. Each core computes, for its
batch b and its half of the query pixels (n), the full attention output
gamma * (V @ softmax(Q^T K)^T) + x over all m=4096 keys. The host permutes
each core's pixel columns so the core's own query half sits first (attention
is permutation-invariant over keys), so every core runs the same program.

Design notes (all timings from the CoreSim cost model this target runs on):

  - Quadratic-form energy: e[n,m] = x_aug_n^T M x_aug_m with
    M = [[Wq^T Wk, Wq^T bk], [bq^T Wk, bq^T bk]] / 64 folded host-side.
    On-device setup is one z = M^T x_aug pass (8 matmuls + 8 PSUM->SBUF
    copies); energy blocks are then zt-stationary x_aug-moving matmuls, so
    the moving operand comes straight from the DMA'd input (no q/k copies).

  - The exp stream is split across BOTH elementwise engines:
      * ACT: exp(64*u) via activation scale=64 (exact table exp),
        ~1004ns per 1024-elem group.
      * DVE: two registered custom-DVE ops (registered into
        concourse.dve_ops at import): EXPG_P1 = deg-4 Horner poly for
        exp(u) (coeffs fitted on |e|<=24, c0=c1=1), EXPG_P2 = six chained
        squarings -> exp(u)^64 = exp(e). Max rel err ~4e-4 over the actual
        energy range (|e| < 17), below the bf16 output rounding.
    Groups are assigned to engines per-window to balance (ACT ~1.0us per
    group vs DVE ~2.3us; DVE also owns the setup copies + epilogue).

  - AV is TRANSPOSED: oa[n-block 128, 65] += exp(E)[m,n-block]^T @ vT[m,65]
    per m-block, i.e. 65-cycle moving operands (~36ns each, 128/window)
    instead of 512-cycle [65,512] accumulations (~216ns, 32/window).
    Stationary loads are not charged by the cost model. Column 64 of vT is
    ones -> oa column 64 accumulates the softmax denominator.
    All 4 accumulators of a window pack into ONE PSUM bank; only the very
    first AV matmul of a window uses start=True (start zeroes the whole
    2KB bank), everything else accumulates.

  - Epilogue per window: reciprocal_approx_fast on the 4 denominators
    (strided [128,4] view), then one affine_then_add per n-block
    (out = oa*rec + xres), DMA out in [n, c] layout (host untransposes).

  - PSUM: 8 banks = energy 2x[128,1024] + oa 2x[128,260] + setup/warmup
    2x[128,512]. PE clock ramps only after ~3us of continuous busy; a short
    warmup stream bridges the z0-copy wait at the head.
"""

import os
import sys
import types
import numpy as np
import ml_dtypes


def _ensure_ntff_hook_importable():
    """bass_utils imports antenv.axon_hooks when tracing is requested via
    BASS_TRACE; some images lack that module. Provide it (backed by the
    ctypes hook from trn_boot when available, else a None hook, which
    bass_utils handles by skipping the trace)."""
    try:
        import antenv.axon_hooks  # noqa: F401
        return
    except ImportError:
        pass
    hook = None
    try:
        from trn_agent_boot.trn_boot import _ntff_profile_via_ctypes
        so = "/opt/axon/libaxon_pjrt.so"
        if os.path.exists(so):
            hook = _ntff_profile_via_ctypes(so)
    except Exception:
        hook = None
    mod = types.ModuleType("antenv.axon_hooks")
    mod.get_axon_ntff_profile_hook = lambda: hook
    sys.modules["antenv.axon_hooks"] = mod


B, C, H, W = 4, 64, 64, 64
N = H * W              # 4096 pixels
NHALF = N // 2         # 2048 query pixels per core
NCORES = 8
MBLK = 128             # m-block (PSUM partition tile)
WIN = 512              # query-window width
NWIN = NHALF // WIN    # 4
NJ = N // MBLK         # 32 m-blocks
GRP = 2                # m-blocks per exp group
NGRP = NJ // GRP       # 16 groups per window
XW = 192               # weights prefix cols in xall (M_padT 128 + Wv_aug 64)
XCOLS = XW + N

ESCALE = 64.0          # energy prescale folded into M host-side
# deg-4 poly coeffs for exp(u) on |u| <= 24/64, c0=c1=1 (fit min rel err)
PC4, PC3, PC2 = 0.04083403291898538, 0.1675708986424995, 0.5000982898691694

# per-window ACT-group counts (rest of the 16 are DVE groups), tunable
ACT_COUNTS = [int(v) for v in os.environ.get("KACT", "14,12,11,12").split(",")]
LAG_ACT = int(os.environ.get("KLAGA", "1"))
LAG_DVE = int(os.environ.get("KLAGD", "3"))
NWARM = int(os.environ.get("KWARM", "0"))

_compiled = {}
LAST_RESULT = None


def _dve_positions(nact, first=False, last=False):
    """Spread the (16 - nact) DVE groups evenly over slots [lo, 13]: slots
    0-1 stay ACT so the window's first AVs flush early, and even spacing
    keeps both exp engines continuously fed (clustering starves one of
    them). Window 0 starts at slot 5 — its early slots are congested with
    the dripped setup pieces, and an early DVE group there holds an eps
    slot hostage."""
    nd = NGRP - nact
    if nd <= 0:
        return set()
    lo, hi = (5, 13) if first else (2, 13)
    pos = set()
    for i in range(nd):
        p = lo + int(round(i * (hi - lo) / max(1, nd - 1))) if nd > 1 \
            else (lo + hi) // 2
        while p in pos:
            p += 1
        pos.add(min(p, NGRP - 1))
    return pos


def _register_exp_ops():
    """Register the two exp custom-DVE ops into concourse.dve_ops so both
    the CoreSim reference execution and the per-NEFF uop-table generation
    (bass_utils.dve_table_for_ops) can see them."""
    import concourse.dve_ops as dops
    from concourse.dve_spec import Spec, Src0, C0, C1, C2, One, lower
    from concourse.dve_uop import DveOpSpec

    def p1_ref(in0, in1, s0, s1, imm2):
        x = in0.astype(np.float32)
        one = np.float32(1.0)
        return ((((x * np.float32(s0) + np.float32(s1)) * x + np.float32(imm2))
                 * x + one) * x + one).astype(np.float32)

    def p2_ref(in0, in1, s0, s1, imm2):
        x = in0.astype(np.float32)
        for _ in range(6):
            x = (x * x).astype(np.float32)
        return x

    u = Src0
    p1_body = ((((u * C0 + C1) * u + C2) * u + One) * u + One)
    t = Src0
    for _ in range(6):
        t = t * t

    ops = {}
    for name, body, ref in [("ANT_EXPG_P1", p1_body, p1_ref),
                            ("ANT_EXPG_P2", t, p2_ref)]:
        existing = next((o for o in dops.OPS if o.name == name), None)
        if existing is not None:
            ops[name] = existing
            continue
        spec = Spec(body=body, reference=ref)
        row = dops._CUSTOM_DVE_ROW_BASE + len(dops.OPS)
        assert row < 0x20, "custom-DVE opcode rows exhausted"
        dops._SUB_OPCODE_FOR_NAME[name] = row
        sha = {}
        for ver in ("v3", "v4"):
            try:
                uops = lower(spec, ver=ver)
                sha[ver] = DveOpSpec(name=name, opcode=row, uops=uops,
                                     rd1_en=False).sha(ver)
            except Exception:
                pass
        op = dops.DveOp(name, spec, subdim=False, uops_sha=sha)
        dops.OPS.append(op)
        dops.CUSTOM_DVE_SPECS[name] = spec
        ops[name] = op
    return ops["ANT_EXPG_P1"], ops["ANT_EXPG_P2"]


def _build():
    import concourse.bacc as bacc
    import concourse.mybir as mybir
    from concourse.tile import TileContext

    EXP_P1, EXP_P2 = _register_exp_ops()

    dt = mybir.dt
    f32, bf16 = dt.float32, dt.bfloat16
    EXP = mybir.ActivationFunctionType.Exp

    nc = bacc.Bacc("TRN2", target_bir_lowering=False, debug=False,
                   num_devices=NCORES)

    xall = nc.dram_tensor("xall", [128, XCOLS], bf16, kind="ExternalInput").ap()
    xres = nc.dram_tensor("xres", [128, 16 * C], f32, kind="ExternalInput").ap()
    outt = nc.dram_tensor("outt", [128, 16 * C], f32, kind="ExternalOutput").ap()

    with TileContext(nc) as tc:
        with tc.tile_pool(name="const", bufs=1) as cp, \
             tc.tile_pool(name="eps", bufs=3, space="PSUM") as eps, \
             tc.tile_pool(name="oap", bufs=2, space="PSUM") as oap, \
             tc.tile_pool(name="wp", bufs=6) as wp, \
             tc.tile_pool(name="pp", bufs=2) as pp, \
             tc.tile_pool(name="fp", bufs=2) as fp:

            # ---- DMA in: weights + window-0 block first, then the rest ----
            # DMA: weights + window-0 queries/first keys, then keys 512:2048
            # (unblocks the z1-z3 setup drips early), then the second half
            # d1 on the SP queue; d2a/d2b on the Activation HWDGE queue so
            # the descriptor generations run concurrently, not serially
            xall_t = cp.tile([128, XCOLS], bf16, tag="xa", name="xall_t")
            nc.sync.dma_start(out=xall_t[:, 0:XW + WIN], in_=xall[:, 0:XW + WIN])
            nc.scalar.dma_start(out=xall_t[:, XW + WIN:XW + 4 * WIN],
                                in_=xall[:, XW + WIN:XW + 4 * WIN])
            nc.scalar.dma_start(out=xall_t[:, XW + 4 * WIN:],
                                in_=xall[:, XW + 4 * WIN:])
            xr_t = cp.tile([128, 16 * C], f32, tag="xr", name="xr_t")

            zt = cp.tile([128, N], bf16, tag="z", name="zt")
            vt = cp.tile([128, NJ * (C + 1)], bf16, tag="vt", name="vt")
            vt3 = vt.rearrange("p (j c) -> p j c", c=C + 1)
            nc.vector.memset(vt3[:, :, C], 1.0)

            if NWARM > 0:
                wu = cp.tile([128, WIN], bf16, tag="wu", name="wu")
                nc.vector.memset(wu[:, :], 0.0)

            x_aug = xall_t[:, XW:]          # [128, 4096] keys/queries
            wv_aug = xall_t[:, 128:XW]      # [128, 64]

            # ---- setup emitters ----
            def emit_z(c):
                # z chunk c: [128, 512] = M_padT^T-applied x_aug columns
                zp = eps.tile([128, GRP * WIN], f32, tag="e", name="zp")
                nc.tensor.matmul(zp[:, 0:WIN], xall_t[:, 0:128],
                                 x_aug[:, WIN * c:WIN * (c + 1)],
                                 start=True, stop=True)
                nc.vector.tensor_copy(zt[:, WIN * c:WIN * (c + 1)],
                                      zp[:, 0:WIN])

            def emit_vt(p8):
                # vT blocks 8p8..8p8+7: out[m,64] per block, one 512-col
                # copy per 8 blocks (all within one PSUM bank; the per-block
                # start=True lazy-zero does not disturb sibling blocks)
                vp = eps.tile([128, GRP * WIN], f32, tag="e", name="vp")
                for jj in range(8):
                    j = 8 * p8 + jj
                    nc.tensor.matmul(vp[:, C * jj:C * (jj + 1)],
                                     x_aug[:, MBLK * j:MBLK * (j + 1)],
                                     wv_aug, start=True, stop=True)
                vp8 = vp.rearrange("p (j c) -> p j c", c=C)
                nc.vector.tensor_copy(vt3[:, 8 * p8:8 * p8 + 8, 0:C],
                                      vp8[:, 0:8, :])

            # eager: z0 + v0 (window 0 group 0/1 needs them)
            emit_z(0)
            emit_vt(0)

            # optional warmup matmuls (into an eps slot) to start the PE
            # clock ramp while the z0 copy is in flight
            if NWARM > 0:
                wu_p = eps.tile([128, GRP * WIN], f32, tag="e", name="wu_p")
                for _ in range(NWARM):
                    nc.tensor.matmul(wu_p[:, 0:WIN], wu[:, 0:128], wu[:, :],
                                     start=True, stop=True)

            # interleaved drip for the remaining setup: z_c before group 2c's
            # energy (emitted at iteration 2c-1), vT piece p (8 blocks)
            # before its AVs flush (~group 4p)
            setup_thunks = [lambda: emit_z(1), lambda: emit_vt(1),
                            lambda: emit_z(2), lambda: emit_z(3),
                            lambda: emit_vt(2), lambda: emit_z(4),
                            lambda: emit_z(5), lambda: emit_vt(3),
                            lambda: emit_z(6), lambda: emit_z(7)]

            # ---- windows ----
            pend_av = []   # (oa, ex, j0, flush_at, first)

            def flush_one(last=False):
                oa_p, ex_p, j0, _fa, first = pend_av.pop(0)
                for jj in range(GRP):
                    for b in range(4):
                        nc.tensor.matmul(
                            oa_p[:, 65 * b:65 * (b + 1)],
                            ex_p[:, 512 * jj + 128 * b:512 * jj + 128 * (b + 1)],
                            vt3[:, j0 + jj, :],
                            start=(first and jj == 0 and b == 0),
                            stop=(last and jj == GRP - 1),
                            skip_group_check=True)

            def epilogue(oa, w):
                oa3 = oa.rearrange("p (b c) -> p b c", c=65)
                rec = fp.tile([128, 4], f32, tag="rec", name="rec")
                nc.vector.reciprocal_approx_fast(out=rec[:, :],
                                                 in_=oa3[:, :, C])
                osb = fp.tile([128, 4 * C], f32, tag="osb", name="osb")
                for b in range(4):
                    nc.vector.affine_then_add(
                        out=osb[:, C * b:C * (b + 1)],
                        in0=oa3[:, b, 0:C],
                        in1=xr_t[:, 4 * C * w + C * b:4 * C * w + C * (b + 1)],
                        scale=rec[:, b:b + 1], bias=0.0)
                nc.sync.dma_start(out=outt[:, 4 * C * w:4 * C * (w + 1)],
                                  in_=osb[:, :])

            energy_tiles = {}

            def emit_energy(g, q_rhs):
                e = eps.tile([128, GRP * WIN], f32, tag="e", name="e")
                for jj in range(GRP):
                    j = GRP * g + jj
                    nc.tensor.matmul(e[:, WIN * jj:WIN * (jj + 1)],
                                     zt[:, MBLK * j:MBLK * (j + 1)],
                                     q_rhs, start=True, stop=True)
                energy_tiles[g] = e

            oa_prev = None
            for w in range(NWIN):
                dve_pos = _dve_positions(ACT_COUNTS[w], first=(w == 0),
                                         last=(w == NWIN - 1))
                oa = oap.tile([128, 4 * 65], f32, tag="oa", name=f"oa{w}")
                q_rhs = x_aug[:, WIN * w:WIN * (w + 1)]
                for g in range(NGRP):
                    is_dve = g in dve_pos
                    # energy runs one group ahead of exp so neither exp
                    # engine ever waits on the in-order PE stream
                    if g == 0:
                        emit_energy(0, q_rhs)
                    if g + 1 < NGRP:
                        emit_energy(g + 1, q_rhs)
                    e = energy_tiles.pop(g)
                    ex = wp.tile([128, GRP * WIN], bf16, tag="ex", name="ex")
                    if is_dve:
                        p = pp.tile([128, GRP * WIN], f32, tag="p", name="p")
                        nc.vector._custom_dve(EXP_P1, out=p[:, :], in0=e[:, :],
                                              s0=PC4, s1=PC3, imm2=PC2)
                        nc.vector._custom_dve(EXP_P2, out=ex[:, :], in0=p[:, :])
                    else:
                        nc.scalar.activation(ex[:, :], e[:, :], EXP,
                                             scale=ESCALE)
                    # drip one setup piece (window 0 only), after exp so the
                    # copy isn't queued ahead of exp work on the DVE
                    if setup_thunks:
                        setup_thunks.pop(0)()
                    pend_av.append((oa, ex, GRP * g,
                                    g + (LAG_DVE if is_dve else LAG_ACT),
                                    g == 0))
                    while pend_av and pend_av[0][3] <= g:
                        flush_one()
                    # window w-1's epilogue once its AVs are all flushed
                    if oa_prev is not None and g == 1:
                        epilogue(*oa_prev)
                        oa_prev = None
                    # the residual DMA is only needed by window 0's epilogue;
                    # issuing it late keeps the input-DMA head short
                    if w == 0 and g == 2:
                        nc.sync.dma_start(out=xr_t[:, :], in_=xres)
                # flush the window's remaining AVs
                while pend_av:
                    flush_one(last=(len(pend_av) == 1))
                oa_prev = (oa, w)
            epilogue(*oa_prev)

    nc.compile()
    return nc


def _get_compiled():
    if "nc" not in _compiled:
        _compiled["nc"] = _build()
    return _compiled["nc"]


def kernel(x, Wq, bq, Wk, bk, Wv, bv, gamma):
    global LAST_RESULT
    _ensure_ntff_hook_importable()
    from concourse.bass_utils import run_bass_kernel_spmd

    nc = _get_compiled()

    x = np.asarray(x, dtype=np.float32)
    xf = x.reshape(B, C, N)
    Wq, Wk, Wv = np.asarray(Wq), np.asarray(Wk), np.asarray(Wv)
    bq, bk, bv = np.asarray(bq), np.asarray(bk), np.asarray(bv)
    gval = float(np.asarray(gamma).reshape(-1)[0])

    # quadratic-form energy matrix (augmented with biases), prescaled by 1/64
    M65 = np.zeros((65, 65), np.float64)
    M65[0:C, 0:C] = Wq.T.astype(np.float64) @ Wk.astype(np.float64)
    M65[0:C, C] = Wq.T.astype(np.float64) @ bk.astype(np.float64)
    M65[C, 0:C] = bq.astype(np.float64) @ Wk.astype(np.float64)
    M65[C, C] = float(bq.astype(np.float64) @ bk.astype(np.float64))
    m_padt = np.zeros((128, 128), np.float32)
    m_padt[0:65, 0:65] = (M65 / ESCALE).T.astype(np.float32)

    wv_aug = np.zeros((128, C), np.float32)
    wv_aug[0:C] = gval * Wv.T
    wv_aug[C] = gval * bv

    in_maps = []
    for core in range(NCORES):
        b, h = divmod(core, 2)
        own = xf[b][:, h * NHALF:(h + 1) * NHALF]
        oth = xf[b][:, (1 - h) * NHALF:(2 - h) * NHALF]
        x_aug = np.zeros((128, N), np.float32)
        x_aug[0:C] = np.concatenate([own, oth], axis=1)
        x_aug[C] = 1.0
        xall_core = np.concatenate([m_padt, wv_aug, x_aug], axis=1)
        # residual in [n-block, c] layout: xres_t[p, 64b+c] = own[c, 128b+p]
        xres_core = np.ascontiguousarray(
            own.reshape(C, 16, 128).transpose(2, 1, 0).reshape(128, 16 * C),
            dtype=np.float32)
        in_maps.append({
            "xall": np.ascontiguousarray(xall_core.astype(ml_dtypes.bfloat16)),
            "xres": xres_core,
        })

    trace = bool(os.environ.get("KTRACE"))
    res = run_bass_kernel_spmd(nc, in_maps, list(range(NCORES)), trace=trace)
    LAST_RESULT = res

    outf = np.empty((B, C, N), dtype=np.float32)
    for core in range(NCORES):
        b, h = divmod(core, 2)
        r = res.results[core]["outt"]          # [128, 16*64]
        half = r.reshape(128, 16, C).transpose(2, 1, 0).reshape(C, NHALF)
        outf[b][:, h * NHALF:(h + 1) * NHALF] = half
    return outf.reshape(B, C, H, W)
